# revision 12
# baseline (speedup 1.0000x reference)
"""Trainium2 Bass kernel for nn_Downsampler: depthwise 4x4 conv, stride 4,
VALID padding, one shared (runtime) 4x4 kernel across all channels.

  x: (16, 8, 1024, 1024) f32, kernel: (4, 4) f32 -> out: (16, 8, 256, 256) f32

Sharding: pure data parallel over batch N=16 -> 2 batches per core on 8 cores.

Math: out[o, j] = sum_{di,dj} k[di,dj] * x[4o+di, 4j+dj], rows flattened over
(n, c, h) since every image row has W=1024 and slabs never straddle an (n, c)
boundary (1024 rows per image, slab = 512 rows).

Two-stage implementation, per slab of 512 input rows held as an SBUF tile
[128, 4096] (partition p, quarter d -> row 512*s + 128*d + p):

1. Horizontal pass (W-downsample). Row r uses kernel row k[r%4, :], and
   r%4 == p%4 in every quarter, so the weights are a per-partition scalar
   ks[p, dj] = kernel[p%4, dj]. FOUR independent tap multiplies (no adds on
   the vector engines -- the PE accumulates all four):
       m_dj[p, (d, j)] = ks[p, dj] * xt[p, (d, 4j+dj)]
   spread: m0 on ScalarE ACTIVATE(Copy, scale), m1 on GpSimd tensor_tensor,
   m2 on VectorE tensor_scalar, m3 alternating ScalarE/VectorE per slab.
   This keeps every engine under the per-slab DMA time (~5.7us) even during
   HAM K=4/8 half-clock windows (HAM gates only the PE clock, but a slow
   engine backs the whole pipeline up into the input DMA stream).

2. Vertical pass (H-downsample) on the TensorEngine with a 0/1 selection
   matrix sel[p, m] = (p//4 == m), contracting the 4 rows of each group in a
   4-deep PSUM accumulation group (one matmul per tap tile):
       psum[m, (d, j)] = sum_p sel[p, m] * (m0+m1+m2+m3)[p, (d, j)]
   Tap tiles and sel are bf16 (0/1 sel values are exact): a bf16 moving
   operand streams the PE at 1 cycle/row (vs 4 for plain fp32), so the PE
   stays far below the DMA pace even when the HAM clock gate throttles it
   to K=4/8 half clock. bf16 tap rounding costs ~2e-3 rel error, well
   inside the 2e-2 gate.

PSUM eviction (VectorE tensor_scalar*1.0, ~1.2us -- DVE reads PSUM cheaper
than ScalarE) and the output DMA for slab s are emitted TAIL_LAG groups
later: engine queues are in-order, so an eagerly-emitted evict(s) would sit
at the head of the queue waiting on matmul(s) and stall slab s+1's work
behind it. The output DMA rides the ScalarE HWDGE ring; the SP ring is a
pure input stream (const loads also ride the ScalarE ring). The first
input-DMA pair is split into 4 quarter DMAs so the first packets hit HBM
sooner (descriptor generation for a full 1024-descriptor batch delays the
first transfer otherwise).

PSUM eviction is split into two free-dim halves on ScalarE + VectorE in
parallel; the final flush's output DMAs ride both HWDGE rings (the input
stream is done by then, so the SP ring is free).
"""

import json
from contextlib import ExitStack

import numpy as np

import concourse.bass as bass
import concourse.mybir as mybir
from concourse.tile import TileContext
from concourse.bass_utils import run_bass_kernel_spmd

N, C, H, W = 16, 8, 1024, 1024
F = 4
N_CORES = 8
R = (N // N_CORES) * C * H  # input rows per core (16384)
WO = W // F  # output row length (256)



def _split_excess_waits(bir_bytes: bytes, max_waits: int = 1) -> bytes:
    """The public neuronxcc walrus supports at most ONE sync wait per
    instruction; hoist excess waits onto NoOps inserted just before."""
    m = json.loads(bir_bytes)

    def fix(blocks):
        for bb in blocks:
            out = []
            for ins in bb.get("instructions", []):
                si = ins.get("sync_info")
                waits = (si or {}).get("on_wait") or []
                if len(waits) > max_waits:
                    extra = waits[:-max_waits]
                    si["on_wait"] = waits[-max_waits:]
                    for i in range(0, len(extra), max_waits):
                        out.append(
                            {
                                "debug": ins.get("debug", 0),
                                "engine": ins["engine"],
                                "ins": [],
                                "outs": [],
                                "name": f"{ins['name']}-ws{i}",
                                "opcode": "NoOp",
                                "sync_info": {
                                    "on_update": [],
                                    "on_wait": extra[i : i + max_waits],
                                },
                            }
                        )
                out.append(ins)
            bb["instructions"] = out
            fix(bb.get("blocks", []))

    for f in m["functions"]:
        fix(f["blocks"])
    return json.dumps(m).encode()


def _make_ks(kernel: np.ndarray) -> np.ndarray:
    """Per-partition horizontal weights [128, 4]: ks[p, dj] = kernel[p%4, dj]."""
    kernel = np.asarray(kernel, dtype=np.float32)
    assert kernel.shape == (F, F)
    return np.ascontiguousarray(kernel[np.arange(128) % F, :])


def _make_sel() -> np.ndarray:
    """Vertical selection matmul weights [128, 32]: sel[p, m] = (p//4 == m).
    bf16: 0/1 are exact, and a bf16 moving/stationary pair runs the PE at
    1 cycle/row (vs 4 for fp32)."""
    import ml_dtypes

    p = np.arange(128)
    return (p[:, None] // F == np.arange(32)[None, :]).astype(ml_dtypes.bfloat16)


def _build_nc(
    rows: int, xt_bufs: int = 3, m_bufs: int = 6, psum_bufs: int = 3, o_bufs: int = 4
) -> bass.Bass:
    assert rows % 2048 == 0
    n_groups = rows // 2048  # 4 slabs of 512 rows per PSUM group

    nc = bass.Bass("TRN2", target_bir_lowering=False, debug=False)
    x = nc.dram_tensor("x", [rows, W], mybir.dt.float32, kind="ExternalInput")
    ks = nc.dram_tensor("ks", [128, F], mybir.dt.float32, kind="ExternalInput")
    sel = nc.dram_tensor("sel", [128, 32], mybir.dt.bfloat16, kind="ExternalInput")
    y = nc.dram_tensor("y", [rows // F, WO], mybir.dt.float32, kind="ExternalOutput")

    mult = mybir.AluOpType.mult

    with TileContext(nc) as tc:
        with ExitStack() as ctx:
            const_pool = ctx.enter_context(tc.tile_pool(name="const_pool", bufs=1))
            kst = const_pool.tile([128, F], mybir.dt.float32)
            nc.scalar.dma_start(kst[:], ks.ap())
            selt = const_pool.tile([128, 32], mybir.dt.bfloat16)
            nc.scalar.dma_start(selt[:], sel.ap())

            x_pool = ctx.enter_context(tc.tile_pool(name="x_pool", bufs=xt_bufs))
            m_pool = ctx.enter_context(tc.tile_pool(name="m_pool", bufs=m_bufs))
            ps_pool = ctx.enter_context(
                tc.tile_pool(name="ps_pool", bufs=psum_bufs, space="PSUM")
            )
            o_pool = ctx.enter_context(tc.tile_pool(name="o_pool", bufs=o_bufs))

            TAIL_LAG = 1  # groups (4 slabs each)
            pending: list = []

            def emit_tail(g: int, pt, final: bool = False) -> None:
                # evict 4 slabs' PSUM -> SBUF at once (DMA cannot read PSUM;
                # GpSimd cannot touch PSUM at all), split into free-dim
                # halves on ScalarE + VectorE so they run in parallel
                ot = o_pool.tile([128, 4 * WO], mybir.dt.float32, name="ot")
                nc.scalar.copy(ot[:, 0:512], pt[:, 0:512])
                nc.vector.tensor_scalar(
                    ot[:, 512:1024], pt[:, 512:1024], 1.0, None, mult
                )
                # ot[32q+m, (d, j)] -> y row (4g+q)*128 + 32*d + m, one DMA
                # per slab (the AP balancer caps at 3 dims). The output DMAs
                # ride the ScalarE HWDGE ring (SP ring stays a pure input
                # stream).
                for q in range(4):
                    base = (4 * g + q) * 128
                    dst = y.ap()[base : base + 128, :].rearrange(
                        "(d m) j -> m d j", d=4
                    )
                    # the final flush may use the SP ring too: the input
                    # stream is done, so outputs no longer serialize behind
                    # input transfers on that queue
                    ring = nc.sync if final and q % 2 else nc.scalar
                    ring.dma_start(
                        dst,
                        ot[32 * q : 32 * q + 32, :].rearrange(
                            "m (d j) -> m d j", d=4
                        ),
                    )

            for g in range(n_groups):
                # one PSUM tile holds 4 slabs via matmul col-tiling: slab
                # q's output lands on partitions 32q..32q+32
                pt = ps_pool.tile([128, 4 * WO], mybir.dt.float32, name="pt")
                for q in range(4):
                    s = 4 * g + q
                    if q % 2 == 0:
                        # one input DMA covers TWO slabs (fewer trigger
                        # gaps in the SP input stream)
                        xt2 = x_pool.tile(
                            [128, 8 * W], mybir.dt.float32, name="xt"
                        )
                        if s == 0:
                            # quarter the very first pair: the first packets
                            # reach HBM after ~1/4 of the descriptor batch
                            # instead of the whole 1024-descriptor batch
                            for k in range(4):
                                r0 = s * 512 + k * 256
                                src = x.ap()[r0 : r0 + 256, :].rearrange(
                                    "(d p) w -> p d w", p=128
                                )
                                nc.sync.dma_start(
                                    xt2[:].rearrange("p (d w) -> p d w", d=8)[
                                        :, 2 * k : 2 * k + 2, :
                                    ],
                                    src,
                                )
                        elif s == rows // 512 - 2:
                            # split the last pair per slab: slab 30 lands
                            # ~2.8us before slab 31, so its taps drain off
                            # the engines while slab 31 is still in flight
                            for k in range(2):
                                r0 = (s + k) * 512
                                src = x.ap()[r0 : r0 + 512, :].rearrange(
                                    "(d p) w -> p d w", p=128
                                )
                                nc.sync.dma_start(
                                    xt2[:].rearrange("p (d w) -> p d w", d=8)[
                                        :, 4 * k : 4 * k + 4, :
                                    ],
                                    src,
                                )
                        else:
                            src = x.ap()[s * 512 : (s + 2) * 512, :].rearrange(
                                "(d p) w -> p d w", p=128
                            )
                            nc.sync.dma_start(
                                xt2[:].rearrange("p (d w) -> p d w", d=8), src
                            )
                    half = q % 2
                    # [128, d, j, dj]: element = xt[p, d*W + 4j + dj]
                    xv = xt2[:].rearrange("p (d j q) -> p d j q", d=8, q=F)[
                        :, 4 * half : 4 * half + 4, :, :
                    ]

                    mt = [
                        m_pool.tile([128, 4 * WO], mybir.dt.bfloat16, name=f"m{i}")
                        for i in range(4)
                    ]
                    mv = [
                        m[:].rearrange("p (d j) -> p d j", d=4) for m in mt
                    ]

                    # four independent tap multiplies, one per engine slot
                    nc.scalar.activation(
                        mv[0], xv[:, :, :, 0],
                        mybir.ActivationFunctionType.Copy, scale=kst[:, 0:1],
                    )
                    nc.gpsimd.tensor_tensor(
                        mv[1],
                        xv[:, :, :, 1],
                        kst[:, 1:2].broadcast_to([128, 4, WO]),
                        mult,
                    )
                    nc.vector.tensor_scalar(
                        mv[2], xv[:, :, :, 2], kst[:, 2:3], None, mult
                    )
                    if s % 2 == 0:
                        nc.scalar.activation(
                            mv[3], xv[:, :, :, 3],
                            mybir.ActivationFunctionType.Copy, scale=kst[:, 3:4],
                        )
                    else:
                        nc.vector.tensor_scalar(
                            mv[3], xv[:, :, :, 3], kst[:, 3:4], None, mult
                        )

                    # vertical pass: 4-deep accumulating fp32r matmul group
                    # contracts sel over the row groups while summing the 4
                    # tap tiles; psum[32q+m, (d,j)] = out row 32d+m of slab s
                    for c in range(2):
                        cs = slice(c * 512, (c + 1) * 512)
                        for i in range(4):
                            nc.tensor.matmul(
                                pt[32 * q : 32 * q + 32, cs],
                                selt[:],
                                mt[i][:, cs],
                                start=(i == 0),
                                stop=(i == 3),
                                tile_position=(0, 32 * q),
                            )
                pending.append((g, pt))
                if len(pending) > TAIL_LAG:
                    pg, ppt = pending.pop(0)
                    emit_tail(pg, ppt)

            for pg, ppt in pending:
                emit_tail(pg, ppt, final=True)

    # walrus 1-wait-per-instruction workaround, applied at serialization time
    orig = nc.to_json_bytes
    nc.to_json_bytes = lambda: _split_excess_waits(orig())
    return nc


_NC_CACHE: dict[int, bass.Bass] = {}


def _get_nc(rows: int = R) -> bass.Bass:
    if rows not in _NC_CACHE:
        _NC_CACHE[rows] = _build_nc(rows)
    return _NC_CACHE[rows]


def run_spmd(x: np.ndarray, kern: np.ndarray, **spmd_kwargs):
    """Shard, run on 8 cores, gather. Returns (output, BassKernelResults)."""
    assert x.shape == (N, C, H, W) and kern.shape == (F, F)
    x = np.ascontiguousarray(x, dtype=np.float32)
    ks = _make_ks(kern)
    sel = _make_sel()
    nb = N // N_CORES
    in_maps = [
        {"x": x[i * nb : (i + 1) * nb].reshape(R, W), "ks": ks, "sel": sel}
        for i in range(N_CORES)
    ]
    nc = _get_nc()
    res = run_bass_kernel_spmd(
        nc, in_maps, core_ids=list(range(N_CORES)), **spmd_kwargs
    )
    out = np.concatenate(
        [res.results[i]["y"].reshape(nb, C, H // F, WO) for i in range(N_CORES)],
        axis=0,
    )
    return out, res


def kernel(x: np.ndarray, kernel: np.ndarray) -> np.ndarray:
    out, _ = run_spmd(x, kernel)
    return out


# revision 13
# speedup vs baseline: 1.1130x; 1.1130x over previous
"""Trainium2 Bass kernel for nn_Downsampler: depthwise 4x4 conv, stride 4,
VALID padding, one shared (runtime) 4x4 kernel across all channels.

  x: (16, 8, 1024, 1024) f32, kernel: (4, 4) f32 -> out: (16, 8, 256, 256) f32

Sharding: pure data parallel over batch N=16 -> 2 batches per core on 8 cores.

Math: out[o, j] = sum_{di,dj} k[di,dj] * x[4o+di, 4j+dj], rows flattened over
(n, c, h) since every image row has W=1024 and slabs never straddle an (n, c)
boundary (1024 rows per image, slab = 512 rows).

Two-stage implementation, per slab of 512 input rows held as an SBUF tile
[128, 4096] (partition p, quarter d -> row 512*s + 128*d + p):

1. Horizontal pass (W-downsample). Row r uses kernel row k[r%4, :], and
   r%4 == p%4 in every quarter, so the weights are a per-partition scalar
   ks[p, dj] = kernel[p%4, dj]. FOUR independent tap multiplies (no adds on
   the vector engines -- the PE accumulates all four):
       m_dj[p, (d, j)] = ks[p, dj] * xt[p, (d, 4j+dj)]
   spread: m0 on ScalarE ACTIVATE(Copy, scale), m1 on GpSimd tensor_tensor,
   m2 on VectorE tensor_scalar, m3 alternating ScalarE/VectorE per slab.
   This keeps every engine under the per-slab DMA time (~5.7us) even during
   HAM K=4/8 half-clock windows (HAM gates only the PE clock, but a slow
   engine backs the whole pipeline up into the input DMA stream).

2. Vertical pass (H-downsample) on the TensorEngine with a 0/1 selection
   matrix sel[p, m] = (p//4 == m), contracting the 4 rows of each group in a
   4-deep PSUM accumulation group (one matmul per tap tile):
       psum[m, (d, j)] = sum_p sel[p, m] * (m0+m1+m2+m3)[p, (d, j)]
   Tap tiles and sel are bf16 (0/1 sel values are exact): a bf16 moving
   operand streams the PE at 1 cycle/row (vs 4 for plain fp32), so the PE
   stays far below the DMA pace even when the HAM clock gate throttles it
   to K=4/8 half clock. bf16 tap rounding costs ~2e-3 rel error, well
   inside the 2e-2 gate.

PSUM eviction (VectorE tensor_scalar*1.0, ~1.2us -- DVE reads PSUM cheaper
than ScalarE) and the output DMA for slab s are emitted TAIL_LAG groups
later: engine queues are in-order, so an eagerly-emitted evict(s) would sit
at the head of the queue waiting on matmul(s) and stall slab s+1's work
behind it. The output DMA rides the ScalarE HWDGE ring; the SP ring is a
pure input stream (const loads also ride the ScalarE ring). The first
input-DMA pair is split into 4 quarter DMAs so the first packets hit HBM
sooner (descriptor generation for a full 1024-descriptor batch delays the
first transfer otherwise).

PSUM eviction is split into two free-dim halves on ScalarE + VectorE in
parallel; the final flush's output DMAs ride both HWDGE rings (the input
stream is done by then, so the SP ring is free).
"""

import json
from contextlib import ExitStack

import numpy as np

import concourse.bass as bass
import concourse.mybir as mybir
from concourse.tile import TileContext
from concourse.bass_utils import run_bass_kernel_spmd

N, C, H, W = 16, 8, 1024, 1024
F = 4
N_CORES = 8
R = (N // N_CORES) * C * H  # input rows per core (16384)
WO = W // F  # output row length (256)



def _split_excess_waits(bir_bytes: bytes, max_waits: int = 1) -> bytes:
    """The public neuronxcc walrus supports at most ONE sync wait per
    instruction; hoist excess waits onto NoOps inserted just before."""
    m = json.loads(bir_bytes)

    def fix(blocks):
        for bb in blocks:
            out = []
            for ins in bb.get("instructions", []):
                si = ins.get("sync_info")
                waits = (si or {}).get("on_wait") or []
                if len(waits) > max_waits:
                    extra = waits[:-max_waits]
                    si["on_wait"] = waits[-max_waits:]
                    for i in range(0, len(extra), max_waits):
                        out.append(
                            {
                                "debug": ins.get("debug", 0),
                                "engine": ins["engine"],
                                "ins": [],
                                "outs": [],
                                "name": f"{ins['name']}-ws{i}",
                                "opcode": "NoOp",
                                "sync_info": {
                                    "on_update": [],
                                    "on_wait": extra[i : i + max_waits],
                                },
                            }
                        )
                out.append(ins)
            bb["instructions"] = out
            fix(bb.get("blocks", []))

    for f in m["functions"]:
        fix(f["blocks"])
    return json.dumps(m).encode()


def _make_ks(kernel: np.ndarray) -> np.ndarray:
    """Per-partition horizontal weights [128, 4]: ks[p, dj] = kernel[p%4, dj]."""
    kernel = np.asarray(kernel, dtype=np.float32)
    assert kernel.shape == (F, F)
    return np.ascontiguousarray(kernel[np.arange(128) % F, :])


def _make_sel() -> np.ndarray:
    """Vertical selection matmul weights [128, 32]: sel[p, m] = (p//4 == m).
    bf16: 0/1 are exact, and a bf16 moving/stationary pair runs the PE at
    1 cycle/row (vs 4 for fp32)."""
    import ml_dtypes

    p = np.arange(128)
    return (p[:, None] // F == np.arange(32)[None, :]).astype(ml_dtypes.bfloat16)


def _build_nc(
    rows: int, xt_bufs: int = 3, m_bufs: int = 6, psum_bufs: int = 3, o_bufs: int = 4
) -> bass.Bass:
    assert rows % 2048 == 0
    n_groups = rows // 2048  # 4 slabs of 512 rows per PSUM group

    nc = bass.Bass("TRN2", target_bir_lowering=False, debug=False)
    x = nc.dram_tensor("x", [rows, W], mybir.dt.float32, kind="ExternalInput")
    ks = nc.dram_tensor("ks", [128, F], mybir.dt.float32, kind="ExternalInput")
    sel = nc.dram_tensor("sel", [128, 32], mybir.dt.bfloat16, kind="ExternalInput")
    y = nc.dram_tensor("y", [rows // F, WO], mybir.dt.float32, kind="ExternalOutput")

    mult = mybir.AluOpType.mult

    with TileContext(nc) as tc:
        with ExitStack() as ctx:
            const_pool = ctx.enter_context(tc.tile_pool(name="const_pool", bufs=1))
            kst = const_pool.tile([128, F], mybir.dt.float32)
            nc.scalar.dma_start(kst[:], ks.ap())
            selt = const_pool.tile([128, 32], mybir.dt.bfloat16)
            nc.scalar.dma_start(selt[:], sel.ap())

            # keep-warm scratch: a dummy matmul per slab keeps the PE (and
            # with it the core's activity-managed clock state) busy; every
            # traced run WITHOUT these sustains only ~320 GB/s input DMA vs
            # ~380 GB/s with them (the K=4/8 throttle windows grow and the
            # DMA-to-SBUF path slows), so they pay for themselves ~10x over
            wp_pool = ctx.enter_context(
                tc.tile_pool(name="wp_pool", bufs=1, space="PSUM")
            )
            warm_pt = wp_pool.tile([32, 512], mybir.dt.float32)
            warm_src = const_pool.tile([128, 256], mybir.dt.bfloat16)
            nc.vector.memset(warm_src[:], 1.0)

            x_pool = ctx.enter_context(tc.tile_pool(name="x_pool", bufs=xt_bufs))
            m_pool = ctx.enter_context(tc.tile_pool(name="m_pool", bufs=m_bufs))
            ps_pool = ctx.enter_context(
                tc.tile_pool(name="ps_pool", bufs=psum_bufs, space="PSUM")
            )
            o_pool = ctx.enter_context(tc.tile_pool(name="o_pool", bufs=o_bufs))

            TAIL_LAG = 1  # groups (4 slabs each)
            pending: list = []

            def emit_tail(g: int, pt, final: bool = False) -> None:
                # evict 4 slabs' PSUM -> SBUF at once (DMA cannot read PSUM;
                # GpSimd cannot touch PSUM at all), split into free-dim
                # halves on ScalarE + VectorE so they run in parallel
                ot = o_pool.tile([128, 4 * WO], mybir.dt.float32, name="ot")
                nc.scalar.copy(ot[:, 0:512], pt[:, 0:512])
                nc.vector.tensor_scalar(
                    ot[:, 512:1024], pt[:, 512:1024], 1.0, None, mult
                )
                # ot[32q+m, (d, j)] -> y row (4g+q)*128 + 32*d + m, one DMA
                # per slab (the AP balancer caps at 3 dims). The output DMAs
                # ride the ScalarE HWDGE ring (SP ring stays a pure input
                # stream).
                for q in range(4):
                    base = (4 * g + q) * 128
                    dst = y.ap()[base : base + 128, :].rearrange(
                        "(d m) j -> m d j", d=4
                    )
                    # the final flush may use the SP ring too: the input
                    # stream is done, so outputs no longer serialize behind
                    # input transfers on that queue
                    ring = nc.sync if final and q % 2 else nc.scalar
                    ring.dma_start(
                        dst,
                        ot[32 * q : 32 * q + 32, :].rearrange(
                            "m (d j) -> m d j", d=4
                        ),
                    )

            for g in range(n_groups):
                # one PSUM tile holds 4 slabs via matmul col-tiling: slab
                # q's output lands on partitions 32q..32q+32
                pt = ps_pool.tile([128, 4 * WO], mybir.dt.float32, name="pt")
                for q in range(4):
                    s = 4 * g + q
                    if q % 2 == 0:
                        # one input DMA covers TWO slabs (fewer trigger
                        # gaps in the SP input stream)
                        xt2 = x_pool.tile(
                            [128, 8 * W], mybir.dt.float32, name="xt"
                        )
                        if s == 0:
                            # quarter the very first pair: the first packets
                            # reach HBM after ~1/4 of the descriptor batch
                            # instead of the whole 1024-descriptor batch
                            for k in range(4):
                                r0 = s * 512 + k * 256
                                src = x.ap()[r0 : r0 + 256, :].rearrange(
                                    "(d p) w -> p d w", p=128
                                )
                                nc.sync.dma_start(
                                    xt2[:].rearrange("p (d w) -> p d w", d=8)[
                                        :, 2 * k : 2 * k + 2, :
                                    ],
                                    src,
                                )
                        elif s == rows // 512 - 2:
                            # split the last pair per slab: slab 30 lands
                            # ~2.8us before slab 31, so its taps drain off
                            # the engines while slab 31 is still in flight
                            for k in range(2):
                                r0 = (s + k) * 512
                                src = x.ap()[r0 : r0 + 512, :].rearrange(
                                    "(d p) w -> p d w", p=128
                                )
                                nc.sync.dma_start(
                                    xt2[:].rearrange("p (d w) -> p d w", d=8)[
                                        :, 4 * k : 4 * k + 4, :
                                    ],
                                    src,
                                )
                        else:
                            src = x.ap()[s * 512 : (s + 2) * 512, :].rearrange(
                                "(d p) w -> p d w", p=128
                            )
                            nc.sync.dma_start(
                                xt2[:].rearrange("p (d w) -> p d w", d=8), src
                            )
                    half = q % 2
                    # [128, d, j, dj]: element = xt[p, d*W + 4j + dj]
                    xv = xt2[:].rearrange("p (d j q) -> p d j q", d=8, q=F)[
                        :, 4 * half : 4 * half + 4, :, :
                    ]

                    mt = [
                        m_pool.tile([128, 4 * WO], mybir.dt.bfloat16, name=f"m{i}")
                        for i in range(4)
                    ]
                    mv = [
                        m[:].rearrange("p (d j) -> p d j", d=4) for m in mt
                    ]

                    # four independent tap multiplies, one per engine slot
                    nc.scalar.activation(
                        mv[0], xv[:, :, :, 0],
                        mybir.ActivationFunctionType.Copy, scale=kst[:, 0:1],
                    )
                    nc.gpsimd.tensor_tensor(
                        mv[1],
                        xv[:, :, :, 1],
                        kst[:, 1:2].broadcast_to([128, 4, WO]),
                        mult,
                    )
                    nc.vector.tensor_scalar(
                        mv[2], xv[:, :, :, 2], kst[:, 2:3], None, mult
                    )
                    if s % 2 == 0:
                        nc.scalar.activation(
                            mv[3], xv[:, :, :, 3],
                            mybir.ActivationFunctionType.Copy, scale=kst[:, 3:4],
                        )
                    else:
                        nc.vector.tensor_scalar(
                            mv[3], xv[:, :, :, 3], kst[:, 3:4], None, mult
                        )

                    # vertical pass: 4-deep accumulating fp32r matmul group
                    # contracts sel over the row groups while summing the 4
                    # tap tiles; psum[32q+m, (d,j)] = out row 32d+m of slab s
                    for c in range(2):
                        cs = slice(c * 512, (c + 1) * 512)
                        for i in range(4):
                            nc.tensor.matmul(
                                pt[32 * q : 32 * q + 32, cs],
                                selt[:],
                                mt[i][:, cs],
                                start=(i == 0),
                                stop=(i == 3),
                                tile_position=(0, 32 * q),
                            )
                    # keep-warm dummy (result never read)
                    nc.tensor.matmul(
                        warm_pt[:, 0:256],
                        selt[:],
                        warm_src[:],
                        start=True,
                        stop=True,
                    )

                pending.append((g, pt))
                if len(pending) > TAIL_LAG:
                    pg, ppt = pending.pop(0)
                    emit_tail(pg, ppt)

            for pg, ppt in pending:
                emit_tail(pg, ppt, final=True)

    # walrus 1-wait-per-instruction workaround, applied at serialization time
    orig = nc.to_json_bytes
    nc.to_json_bytes = lambda: _split_excess_waits(orig())
    return nc


_NC_CACHE: dict[int, bass.Bass] = {}


def _get_nc(rows: int = R) -> bass.Bass:
    if rows not in _NC_CACHE:
        _NC_CACHE[rows] = _build_nc(rows)
    return _NC_CACHE[rows]


def run_spmd(x: np.ndarray, kern: np.ndarray, **spmd_kwargs):
    """Shard, run on 8 cores, gather. Returns (output, BassKernelResults)."""
    assert x.shape == (N, C, H, W) and kern.shape == (F, F)
    x = np.ascontiguousarray(x, dtype=np.float32)
    ks = _make_ks(kern)
    sel = _make_sel()
    nb = N // N_CORES
    in_maps = [
        {"x": x[i * nb : (i + 1) * nb].reshape(R, W), "ks": ks, "sel": sel}
        for i in range(N_CORES)
    ]
    nc = _get_nc()
    res = run_bass_kernel_spmd(
        nc, in_maps, core_ids=list(range(N_CORES)), **spmd_kwargs
    )
    out = np.concatenate(
        [res.results[i]["y"].reshape(nb, C, H // F, WO) for i in range(N_CORES)],
        axis=0,
    )
    return out, res


def kernel(x: np.ndarray, kernel: np.ndarray) -> np.ndarray:
    out, _ = run_spmd(x, kernel)
    return out
